# revision 51
# baseline (speedup 1.0000x reference)
"""Trainium2 Bass kernel for GQA attention (B=1, S=2048, D=2048, 32 Q heads,
8 KV heads, head_dim 64), 8-way tensor parallel over heads.

Strategy (SPMD, one graph on all 8 cores):
  - Core c owns Q heads 4c..4c+3 and KV head c (GQA maps exactly).
  - Host prep: x is transposed to model-dim-major bf16 (xT), weight slices are
    cast to bf16; RoPE pairs are de-interleaved via weight-column permutation so
    the rotation becomes two contiguous 32-row blocks; cos/sin tables and
    multiplicative mask tiles are prebuilt from the actual mask input.
  - Device: QKV projections -> RoPE -> scores S^T[k,q] = K^T Q per 128-k' tile
    (causally skipping fully-masked tiles) -> exp on ScalarE (scale 1/8) ->
    multiplicative mask on partial tiles -> attn^T = [V|1]^T P^T accumulated in
    PSUM (ones column yields the softmax denominator for free) -> scale by
    reciprocal -> AllToAll so each core ends with all 32 heads for its 256
    sequence rows -> out rows = attnT^T @ wo (full wo) -> [256, 2048] f32.
  - Host: concatenate row slices -> [1, 2048, 2048].
"""

import os
import sys

import numpy as np

for _p in ("/opt/trn_rl_repo", "/root/.axon_site/_ro/trn_rl_repo"):
    if os.path.isdir(_p) and _p not in sys.path:
        sys.path.insert(0, _p)

import ml_dtypes  # noqa: E402

from concourse import bacc, mybir, tile  # noqa: E402
from concourse.bass_utils import run_bass_kernel_spmd  # noqa: E402

BF16 = mybir.dt.bfloat16
F32 = mybir.dt.float32

S = 2048          # sequence length
D = 2048          # model dim
HD = 64           # head dim
NH = 32           # query heads
NKV = 8           # kv heads
NC = 8            # cores
HL = NH // NC     # q heads per core = 4
P = 128
QG = 512          # q-group width (score-tile free dim)
NG = S // QG      # 4 q groups
NT = S // P       # 16 k'-tiles
KD = D // P       # 16 contraction tiles for D-reductions
SR = S // NC      # 256 output rows per core

_bf = ml_dtypes.bfloat16


def _classify_mask(mask):
    """Per (q-group g, k'-tile t): 'full' (all pass), 'skip' (all blocked), or a
    unique multiplicative-mask tile index. Mask is additive (reference adds it
    to scores before softmax); exp(mask) turns it multiplicative."""
    mexp = np.exp(np.minimum(mask.astype(np.float64), 50.0)).astype(np.float32).T  # [k, q]
    kinds = {}
    uniq = []
    uniq_keys = {}
    for g in range(NG):
        for t in range(NT):
            tl = mexp[P * t:P * (t + 1), QG * g:QG * (g + 1)]
            if np.all(tl == 1.0):
                kinds[(g, t)] = ("full", None)
            elif np.all(tl == 0.0):
                kinds[(g, t)] = ("skip", None)
            else:
                key = tl.tobytes()
                if key not in uniq_keys:
                    uniq_keys[key] = len(uniq)
                    uniq.append(tl.astype(_bf))
                kinds[(g, t)] = ("mul", uniq_keys[key])
    return kinds, uniq


def _build_nc(kinds, n_uniq, dbg=False):
    nc = bacc.Bacc("TRN2", target_bir_lowering=False, debug=False,
                   num_devices=NC)

    xt_d = nc.dram_tensor("xt", [D, S], BF16, kind="ExternalInput")
    wq_d = nc.dram_tensor("wq", [D, HL * HD], BF16, kind="ExternalInput")
    wkv_d = nc.dram_tensor("wkv", [D, 2 * HD], BF16, kind="ExternalInput")
    wo_d = nc.dram_tensor("wo", [D, D], BF16, kind="ExternalInput")
    cos2_d = nc.dram_tensor("cos2", [P, S], BF16, kind="ExternalInput")
    sin2_d = nc.dram_tensor("sin2", [P, S], BF16, kind="ExternalInput")
    ident_d = nc.dram_tensor("ident", [P, P], BF16, kind="ExternalInput")
    mt_d = None
    if n_uniq:
        mt_d = nc.dram_tensor("mtiles", [n_uniq, P, QG], BF16,
                              kind="ExternalInput")
    out_d = nc.dram_tensor("out", [SR, D], F32, kind="ExternalOutput")
    dbg_d = {}
    if dbg:
        dbg_d["qrot"] = nc.dram_tensor("dbg_qrot", [HD, HL, S], BF16,
                                       kind="ExternalOutput")
        dbg_d["krot"] = nc.dram_tensor("dbg_krot", [HD, S], BF16,
                                       kind="ExternalOutput")
        dbg_d["v"] = nc.dram_tensor("dbg_v", [P, NT, HD + 1], BF16,
                                    kind="ExternalOutput")
        dbg_d["asb"] = nc.dram_tensor("dbg_asb", [HL, NG, HD, QG], BF16,
                                      kind="ExternalOutput")
        dbg_d["att"] = nc.dram_tensor("dbg_att", [HL, NG, HD + 1, QG], F32,
                                      kind="ExternalOutput")


    with tile.TileContext(nc) as tc:
        with (
            tc.tile_pool(name="big", bufs=1) as big,          # xt then wo (reused slot)
            tc.tile_pool(name="const", bufs=1) as const,
            tc.tile_pool(name="work", bufs=2) as work,
            tc.tile_pool(name="persist", bufs=1) as persist,
            tc.tile_pool(name="pt", bufs=6) as ptpool,
            tc.tile_pool(name="ps_sc", bufs=3, space="PSUM") as ps_sc,    # [128,1024] f32 = 2 banks ea
            tc.tile_pool(name="ps_attn", bufs=2, space="PSUM") as ps_attn,  # 1 bank ea
            tc.tile_pool(name="dram", bufs=1, space="DRAM") as dram,
        ):
            # ---- inputs. One tile per (k, n-quarter) so dependency tracking
            # lets projection chunk n start once only its slices have landed.
            NQ = S // QG
            xt_sb = [[big.tile([P, QG], BF16, tag=f"bigslot{k}_{n}",
                               name=f"xt{k}_{n}") for n in range(NQ)]
                     for k in range(KD)]
            wq_sb = [const.tile([P, HL * HD], BF16, tag=f"wq{k}", name=f"wq{k}")
                     for k in range(KD)]
            wkv_sb = [const.tile([P, 2 * HD], BF16, tag=f"wkv{k}", name=f"wkv{k}")
                      for k in range(KD)]
            for k in range(KD):
                nc.sync.dma_start(wq_sb[k][:], wq_d.ap()[P * k:P * (k + 1), :])
                nc.sync.dma_start(wkv_sb[k][:], wkv_d.ap()[P * k:P * (k + 1), :])
            for n in range(NQ):
                for k in range(KD):
                    nc.sync.dma_start(
                        xt_sb[k][n][:],
                        xt_d.ap()[P * k:P * (k + 1), QG * n:QG * (n + 1)])

            cos2 = const.tile([P, S], BF16)
            sin2 = const.tile([P, S], BF16)
            ident = const.tile([P, P], BF16)
            nc.sync.dma_start(cos2[:], cos2_d.ap())
            nc.sync.dma_start(sin2[:], sin2_d.ap())
            nc.sync.dma_start(ident[:], ident_d.ap())
            mt = None
            if n_uniq:
                mt = const.tile([P, n_uniq, QG], BF16)
                for u in range(n_uniq):
                    nc.sync.dma_start(mt[:, u, :], mt_d.ap()[u])

            # ---- projections + RoPE + V transpose ----
            qrot = persist.tile([HD, HL, S], BF16)       # [64, 4 heads, 2048]
            krot = persist.tile([HD, S], BF16)
            v_sb = persist.tile([P, NT, HD + 1], BF16)   # [k' part, tile, 65] (col 64 = ones)
            nc.vector.memset(v_sb[:, :, HD:HD + 1], 1.0)

            vt_sb = persist.tile([HD, S], BF16)

            def rope_chunk(raw, nsl):
                # rot = raw*cos2 + blockswap32(raw)*sin2, full 128 partitions
                sw = work.tile([P, QG], BF16, tag="sw")
                for b in range(4):
                    d0, s0 = 32 * b, 32 * (b ^ 1)
                    nc.sync.dma_start(sw[d0:d0 + 32, :], raw[s0:s0 + 32, :])
                t1 = work.tile([P, QG], BF16, tag="t1")
                nc.vector.tensor_mul(t1[:], raw[:], cos2[:, nsl])
                t2 = work.tile([P, QG], BF16, tag="t2")
                nc.vector.tensor_mul(t2[:], sw[:], sin2[:, nsl])
                rot = work.tile([P, QG], BF16, tag="rot")
                nc.vector.tensor_add(rot[:], t1[:], t2[:])
                return rot

            # n-outer, KV first: attention (h, g=n) only needs the first n+1
            # projection column-chunks, so it can start while later chunks load.
            for n in range(S // QG):
                nsl = slice(QG * n, QG * (n + 1))
                for m in (2, 0, 1):  # m=0,1: Q head pairs; m=2: K|V
                    ps = ps_sc.tile([P, QG], F32, tag="scores")
                    for k in range(KD):
                        lhsT = (wq_sb[k][:, P * m:P * (m + 1)] if m < 2
                                else wkv_sb[k][:])
                        nc.tensor.matmul(ps[:], lhsT, xt_sb[k][n][:],
                                         start=(k == 0), stop=(k == KD - 1))
                    raw = work.tile([P, QG], BF16, tag="raw")
                    nc.vector.tensor_copy(raw[:], ps[:])
                    rot = rope_chunk(raw, nsl)
                    if m < 2:
                        nc.vector.tensor_copy(qrot[:, 2 * m, nsl], rot[0:HD, :])
                        nc.sync.dma_start(qrot[:, 2 * m + 1, nsl], rot[HD:P, :])
                    else:
                        nc.vector.tensor_copy(krot[:, nsl], rot[0:HD, :])
                        nc.sync.dma_start(vt_sb[:, nsl], raw[HD:P, :])
                for t in range(4 * n, 4 * n + 4):
                    pv = ps_attn.tile([P, HD], BF16, tag="attn")
                    nc.tensor.transpose(pv[:], vt_sb[:, P * t:P * (t + 1)],
                                        ident[0:HD, 0:HD])
                    nc.vector.tensor_copy(v_sb[:, t, 0:HD], pv[:])

            # ---- attention per (head, q-group) ----
            # Two AllToAlls (heads 0-1, then heads 2-3) so the first one
            # overlaps the second half of attention.
            half = S // NC
            a2a_in = [dram.tile([NC, 2 * HD, half], BF16, tag=f"a2ai{i}",
                                name=f"a2ai{i}") for i in range(2)]
            a2a_out = [dram.tile([NC, 2 * HD, half], BF16, tag=f"a2ao{i}",
                                 name=f"a2ao{i}") for i in range(2)]

            for h in range(HL):
                for g in range(NG):
                    tiles = [t for t in range(NT) if kinds[(g, t)][0] != "skip"]
                    att = ps_attn.tile([HD + 1, QG], F32, tag="attn")
                    qs = slice(QG * g, QG * (g + 1))
                    for ci in range(0, len(tiles), 2):
                        chunk = tiles[ci:ci + 2]
                        psc = ps_sc.tile([P, 2 * QG], F32, tag="scores")
                        for i, t in enumerate(chunk):
                            nc.tensor.matmul(
                                psc[:, QG * i:QG * (i + 1)],
                                krot[:, P * t:P * (t + 1)],
                                qrot[:, h, qs],
                                start=True, stop=True)
                        pts = ptpool.tile([P, 2 * QG], BF16, tag="pt")
                        w = QG * len(chunk)
                        nc.scalar.activation(pts[:, 0:w], psc[:, 0:w],
                                             mybir.ActivationFunctionType.Exp,
                                             scale=0.125)
                        for i, t in enumerate(chunk):
                            kind, u = kinds[(g, t)]
                            if kind == "mul":
                                nc.vector.tensor_mul(
                                    pts[:, QG * i:QG * (i + 1)],
                                    pts[:, QG * i:QG * (i + 1)],
                                    mt[:, u, :])
                        for i, t in enumerate(chunk):
                            nc.tensor.matmul(
                                att[:], v_sb[:, t, :],
                                pts[:, QG * i:QG * (i + 1)],
                                start=(ci + i == 0),
                                stop=(ci + i == len(tiles) - 1))
                    den = work.tile([1, QG], F32, tag="den")
                    nc.scalar.copy(den[:], att[HD:HD + 1, :])
                    rec = work.tile([1, QG], F32, tag="rec")
                    nc.vector.reciprocal_approx_fast(rec[:], den[:])
                    rec64 = work.tile([HD, QG], F32, tag="rec64")
                    nc.gpsimd.partition_broadcast(rec64[:], rec[:])
                    asb = work.tile([HD, QG], BF16, tag="asb")
                    nc.vector.tensor_mul(asb[:], att[0:HD, :], rec64[:])
                    # -> a2ain[j = 2g..2g+1, hd rows of head h, :]
                    buf = a2a_in[h // 2]
                    hr = HD * (h % 2)
                    nc.sync.dma_start(buf[2 * g, hr:hr + HD, :], asb[:, 0:half])
                    nc.sync.dma_start(buf[2 * g + 1, hr:hr + HD, :],
                                      asb[:, half:2 * half])
                    if dbg:
                        nc.sync.dma_start(dbg_d["asb"].ap()[h, g], asb[:])
                        att_f = work.tile([HD + 1, QG], F32, tag="attf")
                        nc.scalar.copy(att_f[:], att[:])
                        nc.sync.dma_start(dbg_d["att"].ap()[h, g], att_f[:])
                if h % 2 == 1:
                    nc.gpsimd.collective_compute(
                        "AllToAll", mybir.AluOpType.bypass,
                        replica_groups=[list(range(NC))],
                        ins=[a2a_in[h // 2].opt()], outs=[a2a_out[h // 2].opt()])

            if dbg:
                nc.sync.dma_start(dbg_d["qrot"].ap(), qrot[:])
                nc.sync.dma_start(dbg_d["krot"].ap(), krot[:])
                nc.sync.dma_start(dbg_d["v"].ap(), v_sb[:])

            # ---- output: out[s_rows, :] = attnT_full^T @ wo ----
            # attnT_full k-tile 2i = a2a_out[0] block i (heads 4i, 4i+1);
            # k-tile 2i+1 = a2a_out[1] block i.
            wo_sb = [[big.tile([P, QG], BF16, tag=f"bigslot{k}_{q}",
                               name=f"wo{k}_{q}") for q in range(D // QG)]
                     for k in range(KD)]
            for k in range(KD):
                for q in range(D // QG):
                    nc.sync.dma_start(
                        wo_sb[k][q][:],
                        wo_d.ap()[P * k:P * (k + 1), QG * q:QG * (q + 1)])
            ao_sb = [persist.tile([P, NC, SR], BF16, tag=f"ao{i}",
                                  name=f"ao{i}") for i in range(2)]
            for i in range(NC):
                nc.sync.dma_start(ao_sb[0][:, i, :], a2a_out[0][i])
                nc.sync.dma_start(ao_sb[1][:, i, :], a2a_out[1][i])

            # Even k-tiles (heads 0-1, from A2A#1) accumulate into SBUF while
            # A2A#2 is in flight; odd k-tiles finish the sum afterwards.
            oacc = persist.tile([P, SR // P, D // QG, QG], F32)
            for sm in range(SR // P):
                for ec in range(D // QG):
                    po = ps_attn.tile([P, QG], F32, tag="attn")
                    for i in range(NC):
                        nc.tensor.matmul(po[:],
                                         ao_sb[0][:, i, P * sm:P * (sm + 1)],
                                         wo_sb[2 * i][ec][:],
                                         start=(i == 0), stop=(i == NC - 1))
                    nc.vector.tensor_copy(oacc[:, sm, ec, :], po[:])
            for sm in range(SR // P):
                for ec in range(D // QG):
                    po = ps_attn.tile([P, QG], F32, tag="attn")
                    for i in range(NC):
                        nc.tensor.matmul(po[:],
                                         ao_sb[1][:, i, P * sm:P * (sm + 1)],
                                         wo_sb[2 * i + 1][ec][:],
                                         start=(i == 0), stop=(i == NC - 1))
                    osb = work.tile([P, QG], F32, tag="osb")
                    nc.vector.tensor_add(osb[:], po[:], oacc[:, sm, ec, :])
                    nc.sync.dma_start(
                        out_d.ap()[P * sm:P * (sm + 1), QG * ec:QG * (ec + 1)],
                        osb[:])

    nc.compile()
    return nc


_CACHE = {}


def _get_compiled(mask):
    kinds, uniq = _classify_mask(mask)
    key = tuple(sorted((k, v[0], v[1]) for k, v in kinds.items()))
    if key not in _CACHE:
        _CACHE[key] = (_build_nc(kinds, len(uniq)), kinds)
    nc, _ = _CACHE[key]
    return nc, kinds, uniq


def _host_prep(x, freqs_cos, freqs_sin, mask, wq, wk, wv, wo, uniq):
    xt = np.ascontiguousarray(x[0].T).astype(_bf)
    perm = np.concatenate([np.arange(0, HD, 2), np.arange(1, HD, 2)])
    cosT = np.ascontiguousarray(freqs_cos.T)            # [32, S]
    sinT = np.ascontiguousarray(freqs_sin.T)
    cos2 = np.tile(cosT, (4, 1)).astype(_bf)            # [128, S]
    sin2 = np.tile(np.concatenate([-sinT, sinT], axis=0), (2, 1)).astype(_bf)
    ident = np.eye(P, dtype=_bf)
    wo_b = np.ascontiguousarray(wo).astype(_bf)
    mt = (np.stack(uniq, axis=0) if uniq
          else np.zeros((0, P, QG), dtype=_bf))

    in_maps = []
    for c in range(NC):
        qcols = np.concatenate(
            [HD * (HL * c + h) + perm for h in range(HL)])
        wq_c = np.ascontiguousarray(wq[:, qcols]).astype(_bf)
        wkv_c = np.concatenate(
            [wk[:, HD * c + perm], wv[:, HD * c:HD * (c + 1)]],
            axis=1).astype(_bf)
        m = {"xt": xt, "wq": wq_c, "wkv": np.ascontiguousarray(wkv_c),
             "wo": wo_b, "cos2": cos2, "sin2": sin2, "ident": ident}
        if len(uniq):
            m["mtiles"] = mt
        in_maps.append(m)
    return in_maps


def run(x, freqs_cos, freqs_sin, mask, wq, wk, wv, wo, trace=False):
    x = np.asarray(x, dtype=np.float32)
    mask = np.asarray(mask, dtype=np.float32)
    nc, kinds, uniq = _get_compiled(mask)
    in_maps = _host_prep(np.asarray(x), np.asarray(freqs_cos),
                         np.asarray(freqs_sin), mask, np.asarray(wq),
                         np.asarray(wk), np.asarray(wv), np.asarray(wo), uniq)
    res = run_bass_kernel_spmd(nc, in_maps, core_ids=list(range(NC)),
                               trace=trace)
    out = np.concatenate([res.results[c]["out"] for c in range(NC)], axis=0)
    return out.reshape(1, S, D).astype(np.float32), res


def kernel(x, freqs_cos, freqs_sin, mask, wq, wk, wv, wo):
    out, _ = run(x, freqs_cos, freqs_sin, mask, wq, wk, wv, wo, trace=False)
    return out


# revision 56
# speedup vs baseline: 1.1828x; 1.1828x over previous
"""Trainium2 Bass kernel for GQA attention (B=1, S=2048, D=2048, 32 Q heads,
8 KV heads, head_dim 64), 8-way tensor parallel over heads.

Strategy (SPMD, one graph on all 8 cores):
  - Core c owns Q heads 4c..4c+3 and KV head c (GQA maps exactly).
  - Host prep: x is transposed to model-dim-major bf16 (xT), weight slices are
    cast to bf16; RoPE pairs are de-interleaved via weight-column permutation so
    the rotation becomes two contiguous 32-row blocks; cos/sin tables and
    multiplicative mask tiles are prebuilt from the actual mask input.
  - Device: QKV projections -> RoPE -> scores S^T[k,q] = K^T Q per 128-k' tile
    (causally skipping fully-masked tiles) -> exp on ScalarE (scale 1/8) ->
    multiplicative mask on partial tiles -> attn^T = [V|1]^T P^T accumulated in
    PSUM (ones column yields the softmax denominator for free) -> scale by
    reciprocal -> AllToAll so each core ends with all 32 heads for its 256
    sequence rows -> out rows = attnT^T @ wo (full wo) -> [256, 2048] f32.
  - Host: concatenate row slices -> [1, 2048, 2048].
"""

import os
import sys

import numpy as np

for _p in ("/opt/trn_rl_repo", "/root/.axon_site/_ro/trn_rl_repo"):
    if os.path.isdir(_p) and _p not in sys.path:
        sys.path.insert(0, _p)

import ml_dtypes  # noqa: E402

from concourse import bacc, mybir, tile  # noqa: E402
from concourse.bass_utils import run_bass_kernel_spmd  # noqa: E402

BF16 = mybir.dt.bfloat16
F32 = mybir.dt.float32

S = 2048          # sequence length
D = 2048          # model dim
HD = 64           # head dim
NH = 32           # query heads
NKV = 8           # kv heads
NC = 8            # cores
HL = NH // NC     # q heads per core = 4
P = 128
QG = 512          # q-group width (score-tile free dim)
NG = S // QG      # 4 q groups
NT = S // P       # 16 k'-tiles
KD = D // P       # 16 contraction tiles for D-reductions
SR = S // NC      # 256 output rows per core

_bf = ml_dtypes.bfloat16


def _classify_mask(mask):
    """Per (q-group g, k'-tile t): 'full' (all pass), 'skip' (all blocked), or a
    unique multiplicative-mask tile index. Mask is additive (reference adds it
    to scores before softmax); exp(mask) turns it multiplicative."""
    mexp = np.exp(np.minimum(mask.astype(np.float64), 50.0)).astype(np.float32).T  # [k, q]
    kinds = {}
    uniq = []
    uniq_keys = {}
    for g in range(NG):
        for t in range(NT):
            tl = mexp[P * t:P * (t + 1), QG * g:QG * (g + 1)]
            if np.all(tl == 1.0):
                kinds[(g, t)] = ("full", None)
            elif np.all(tl == 0.0):
                kinds[(g, t)] = ("skip", None)
            else:
                key = tl.tobytes()
                if key not in uniq_keys:
                    uniq_keys[key] = len(uniq)
                    uniq.append(tl.astype(_bf))
                kinds[(g, t)] = ("mul", uniq_keys[key])
    return kinds, uniq


def _build_nc(kinds, n_uniq, dbg=False):
    nc = bacc.Bacc("TRN2", target_bir_lowering=False, debug=False,
                   num_devices=NC)

    xt_d = nc.dram_tensor("xt", [D, S], BF16, kind="ExternalInput")
    wq_d = nc.dram_tensor("wq", [D, HL * HD], BF16, kind="ExternalInput")
    wkv_d = nc.dram_tensor("wkv", [D, 2 * HD], BF16, kind="ExternalInput")
    wo_d = nc.dram_tensor("wo", [D, D], BF16, kind="ExternalInput")
    cos2_d = nc.dram_tensor("cos2", [P, S], BF16, kind="ExternalInput")
    sin2_d = nc.dram_tensor("sin2", [P, S], BF16, kind="ExternalInput")
    ident_d = nc.dram_tensor("ident", [P, P], BF16, kind="ExternalInput")
    mt_d = None
    if n_uniq:
        mt_d = nc.dram_tensor("mtiles", [n_uniq, P, QG], BF16,
                              kind="ExternalInput")
    out_d = nc.dram_tensor("out", [SR, D], F32, kind="ExternalOutput")
    dbg_d = {}
    if dbg:
        dbg_d["qrot"] = nc.dram_tensor("dbg_qrot", [HD, HL, S], BF16,
                                       kind="ExternalOutput")
        dbg_d["krot"] = nc.dram_tensor("dbg_krot", [HD, S], BF16,
                                       kind="ExternalOutput")
        dbg_d["v"] = nc.dram_tensor("dbg_v", [P, NT, HD + 1], BF16,
                                    kind="ExternalOutput")
        dbg_d["asb"] = nc.dram_tensor("dbg_asb", [HL, NG, HD, QG], BF16,
                                      kind="ExternalOutput")
        dbg_d["att"] = nc.dram_tensor("dbg_att", [HL, NG, HD + 1, QG], F32,
                                      kind="ExternalOutput")


    with tile.TileContext(nc) as tc:
        with (
            tc.tile_pool(name="big", bufs=1) as big,          # xt then wo (reused slot)
            tc.tile_pool(name="const", bufs=1) as const,
            tc.tile_pool(name="work", bufs=2) as work,
            tc.tile_pool(name="persist", bufs=1) as persist,
            tc.tile_pool(name="pt", bufs=6) as ptpool,
            tc.tile_pool(name="ps_sc", bufs=3, space="PSUM") as ps_sc,    # [128,1024] f32 = 2 banks ea
            tc.tile_pool(name="ps_attn", bufs=2, space="PSUM") as ps_attn,  # 1 bank ea
            tc.tile_pool(name="dram", bufs=1, space="DRAM") as dram,
        ):
            # ---- inputs. One tile per k-slice so dependency tracking lets the
            # first projection matmuls start as soon as their tiles land.
            xt_sb = [big.tile([P, S], BF16, tag=f"bigslot{k}", name=f"xt{k}")
                     for k in range(KD)]
            wq_sb = [const.tile([P, HL * HD], BF16, tag=f"wq{k}", name=f"wq{k}")
                     for k in range(KD)]
            wkv_sb = [const.tile([P, 2 * HD], BF16, tag=f"wkv{k}", name=f"wkv{k}")
                      for k in range(KD)]
            for k in range(KD):
                nc.sync.dma_start(xt_sb[k][:], xt_d.ap()[P * k:P * (k + 1), :])
                nc.sync.dma_start(wq_sb[k][:], wq_d.ap()[P * k:P * (k + 1), :])
                nc.sync.dma_start(wkv_sb[k][:], wkv_d.ap()[P * k:P * (k + 1), :])

            cos2 = const.tile([P, S], BF16)
            sin2 = const.tile([P, S], BF16)
            ident = const.tile([P, P], BF16)
            nc.sync.dma_start(cos2[:], cos2_d.ap())
            nc.sync.dma_start(sin2[:], sin2_d.ap())
            nc.sync.dma_start(ident[:], ident_d.ap())
            mt = None
            if n_uniq:
                mt = const.tile([P, n_uniq, QG], BF16)
                for u in range(n_uniq):
                    nc.sync.dma_start(mt[:, u, :], mt_d.ap()[u])

            # ---- projections + RoPE + V transpose ----
            qrot = persist.tile([HD, HL, S], BF16)       # [64, 4 heads, 2048]
            krot = persist.tile([HD, S], BF16)
            v_sb = persist.tile([P, NT, HD + 1], BF16)   # [k' part, tile, 65] (col 64 = ones)
            nc.vector.memset(v_sb[:, :, HD:HD + 1], 1.0)

            vt_sb = persist.tile([HD, S], BF16)

            def rope_chunk(raw, nsl):
                # rot = raw*cos2 + blockswap32(raw)*sin2, full 128 partitions
                sw = work.tile([P, QG], BF16, tag="sw")
                for b in range(4):
                    d0, s0 = 32 * b, 32 * (b ^ 1)
                    nc.sync.dma_start(sw[d0:d0 + 32, :], raw[s0:s0 + 32, :])
                t1 = work.tile([P, QG], BF16, tag="t1")
                nc.vector.tensor_mul(t1[:], raw[:], cos2[:, nsl])
                t2 = work.tile([P, QG], BF16, tag="t2")
                nc.vector.tensor_mul(t2[:], sw[:], sin2[:, nsl])
                rot = work.tile([P, QG], BF16, tag="rot")
                nc.vector.tensor_add(rot[:], t1[:], t2[:])
                return rot

            # n-outer, KV first: attention (h, g=n) only needs the first n+1
            # projection column-chunks, so it can start while later chunks load.
            for n in range(S // QG):
                nsl = slice(QG * n, QG * (n + 1))
                for m in (2, 0, 1):  # m=0,1: Q head pairs; m=2: K|V
                    ps = ps_sc.tile([P, QG], F32, tag="scores")
                    for k in range(KD):
                        lhsT = (wq_sb[k][:, P * m:P * (m + 1)] if m < 2
                                else wkv_sb[k][:])
                        nc.tensor.matmul(ps[:], lhsT, xt_sb[k][:, nsl],
                                         start=(k == 0), stop=(k == KD - 1))
                    raw = work.tile([P, QG], BF16, tag="raw")
                    nc.vector.tensor_copy(raw[:], ps[:])
                    rot = rope_chunk(raw, nsl)
                    if m < 2:
                        nc.vector.tensor_copy(qrot[:, 2 * m, nsl], rot[0:HD, :])
                        nc.sync.dma_start(qrot[:, 2 * m + 1, nsl], rot[HD:P, :])
                    else:
                        nc.vector.tensor_copy(krot[:, nsl], rot[0:HD, :])
                        nc.sync.dma_start(vt_sb[:, nsl], raw[HD:P, :])
                for t in range(4 * n, 4 * n + 4):
                    pv = ps_attn.tile([P, HD], BF16, tag="attn")
                    nc.tensor.transpose(pv[:], vt_sb[:, P * t:P * (t + 1)],
                                        ident[0:HD, 0:HD])
                    nc.vector.tensor_copy(v_sb[:, t, 0:HD], pv[:])

            # ---- attention per (head, q-group) ----
            # Two AllToAlls (heads 0-1, then heads 2-3) so the first one
            # overlaps the second half of attention.
            half = S // NC
            a2a_in = [dram.tile([NC, 2 * HD, half], BF16, tag=f"a2ai{i}",
                                name=f"a2ai{i}") for i in range(2)]
            a2a_out = [dram.tile([NC, 2 * HD, half], BF16, tag=f"a2ao{i}",
                                 name=f"a2ao{i}") for i in range(2)]

            for h in range(HL):
                for g in range(NG):
                    tiles = [t for t in range(NT) if kinds[(g, t)][0] != "skip"]
                    att = ps_attn.tile([HD + 1, QG], F32, tag="attn")
                    qs = slice(QG * g, QG * (g + 1))
                    for ci in range(0, len(tiles), 2):
                        chunk = tiles[ci:ci + 2]
                        psc = ps_sc.tile([P, 2 * QG], F32, tag="scores")
                        for i, t in enumerate(chunk):
                            nc.tensor.matmul(
                                psc[:, QG * i:QG * (i + 1)],
                                krot[:, P * t:P * (t + 1)],
                                qrot[:, h, qs],
                                start=True, stop=True)
                        pts = ptpool.tile([P, 2 * QG], BF16, tag="pt")
                        w = QG * len(chunk)
                        nc.scalar.activation(pts[:, 0:w], psc[:, 0:w],
                                             mybir.ActivationFunctionType.Exp,
                                             scale=0.125)
                        for i, t in enumerate(chunk):
                            kind, u = kinds[(g, t)]
                            if kind == "mul":
                                nc.vector.tensor_mul(
                                    pts[:, QG * i:QG * (i + 1)],
                                    pts[:, QG * i:QG * (i + 1)],
                                    mt[:, u, :])
                        for i, t in enumerate(chunk):
                            nc.tensor.matmul(
                                att[:], v_sb[:, t, :],
                                pts[:, QG * i:QG * (i + 1)],
                                start=(ci + i == 0),
                                stop=(ci + i == len(tiles) - 1))
                    den = work.tile([1, QG], F32, tag="den")
                    nc.scalar.copy(den[:], att[HD:HD + 1, :])
                    rec = work.tile([1, QG], F32, tag="rec")
                    nc.vector.reciprocal_approx_fast(rec[:], den[:])
                    rec64 = work.tile([HD, QG], F32, tag="rec64")
                    nc.gpsimd.partition_broadcast(rec64[:], rec[:])
                    asb = work.tile([HD, QG], BF16, tag="asb")
                    nc.vector.tensor_mul(asb[:], att[0:HD, :], rec64[:])
                    # -> a2ain[j = 2g..2g+1, hd rows of head h, :]
                    buf = a2a_in[h // 2]
                    hr = HD * (h % 2)
                    nc.sync.dma_start(buf[2 * g, hr:hr + HD, :], asb[:, 0:half])
                    nc.sync.dma_start(buf[2 * g + 1, hr:hr + HD, :],
                                      asb[:, half:2 * half])
                    if dbg:
                        nc.sync.dma_start(dbg_d["asb"].ap()[h, g], asb[:])
                        att_f = work.tile([HD + 1, QG], F32, tag="attf")
                        nc.scalar.copy(att_f[:], att[:])
                        nc.sync.dma_start(dbg_d["att"].ap()[h, g], att_f[:])
                if h % 2 == 1:
                    nc.gpsimd.collective_compute(
                        "AllToAll", mybir.AluOpType.bypass,
                        replica_groups=[list(range(NC))],
                        ins=[a2a_in[h // 2].opt()], outs=[a2a_out[h // 2].opt()])

            if dbg:
                nc.sync.dma_start(dbg_d["qrot"].ap(), qrot[:])
                nc.sync.dma_start(dbg_d["krot"].ap(), krot[:])
                nc.sync.dma_start(dbg_d["v"].ap(), v_sb[:])

            # ---- output: out[s_rows, :] = attnT_full^T @ wo ----
            # attnT_full k-tile 2i = a2a_out[0] block i (heads 4i, 4i+1);
            # k-tile 2i+1 = a2a_out[1] block i.
            wo_sb = [big.tile([P, D], BF16, tag=f"bigslot{k}", name=f"wo{k}")
                     for k in range(KD)]
            for k in range(KD):
                nc.sync.dma_start(wo_sb[k][:], wo_d.ap()[P * k:P * (k + 1), :])
            ao_sb = [persist.tile([P, NC, SR], BF16, tag=f"ao{i}",
                                  name=f"ao{i}") for i in range(2)]
            for i in range(NC):
                nc.sync.dma_start(ao_sb[0][:, i, :], a2a_out[0][i])
                nc.sync.dma_start(ao_sb[1][:, i, :], a2a_out[1][i])

            # Even k-tiles (heads 0-1, from A2A#1) accumulate into SBUF while
            # A2A#2 is in flight; odd k-tiles finish the sum afterwards.
            oacc = persist.tile([P, SR // P, D // QG, QG], F32)
            for sm in range(SR // P):
                for ec in range(D // QG):
                    po = ps_attn.tile([P, QG], F32, tag="attn")
                    for i in range(NC):
                        nc.tensor.matmul(po[:],
                                         ao_sb[0][:, i, P * sm:P * (sm + 1)],
                                         wo_sb[2 * i][:, QG * ec:QG * (ec + 1)],
                                         start=(i == 0), stop=(i == NC - 1))
                    nc.vector.tensor_copy(oacc[:, sm, ec, :], po[:])
            for sm in range(SR // P):
                for ec in range(D // QG):
                    po = ps_attn.tile([P, QG], F32, tag="attn")
                    for i in range(NC):
                        nc.tensor.matmul(po[:],
                                         ao_sb[1][:, i, P * sm:P * (sm + 1)],
                                         wo_sb[2 * i + 1][:, QG * ec:QG * (ec + 1)],
                                         start=(i == 0), stop=(i == NC - 1))
                    osb = work.tile([P, QG], F32, tag="osb")
                    nc.vector.tensor_add(osb[:], po[:], oacc[:, sm, ec, :])
                    nc.sync.dma_start(
                        out_d.ap()[P * sm:P * (sm + 1), QG * ec:QG * (ec + 1)],
                        osb[:])

    nc.compile()
    return nc


_CACHE = {}


def _get_compiled(mask):
    kinds, uniq = _classify_mask(mask)
    key = tuple(sorted((k, v[0], v[1]) for k, v in kinds.items()))
    if key not in _CACHE:
        _CACHE[key] = (_build_nc(kinds, len(uniq)), kinds)
    nc, _ = _CACHE[key]
    return nc, kinds, uniq


def _host_prep(x, freqs_cos, freqs_sin, mask, wq, wk, wv, wo, uniq):
    xt = np.ascontiguousarray(x[0].T).astype(_bf)
    perm = np.concatenate([np.arange(0, HD, 2), np.arange(1, HD, 2)])
    cosT = np.ascontiguousarray(freqs_cos.T)            # [32, S]
    sinT = np.ascontiguousarray(freqs_sin.T)
    cos2 = np.tile(cosT, (4, 1)).astype(_bf)            # [128, S]
    sin2 = np.tile(np.concatenate([-sinT, sinT], axis=0), (2, 1)).astype(_bf)
    ident = np.eye(P, dtype=_bf)
    wo_b = np.ascontiguousarray(wo).astype(_bf)
    mt = (np.stack(uniq, axis=0) if uniq
          else np.zeros((0, P, QG), dtype=_bf))

    in_maps = []
    for c in range(NC):
        qcols = np.concatenate(
            [HD * (HL * c + h) + perm for h in range(HL)])
        wq_c = np.ascontiguousarray(wq[:, qcols]).astype(_bf)
        wkv_c = np.concatenate(
            [wk[:, HD * c + perm], wv[:, HD * c:HD * (c + 1)]],
            axis=1).astype(_bf)
        m = {"xt": xt, "wq": wq_c, "wkv": np.ascontiguousarray(wkv_c),
             "wo": wo_b, "cos2": cos2, "sin2": sin2, "ident": ident}
        if len(uniq):
            m["mtiles"] = mt
        in_maps.append(m)
    return in_maps


def run(x, freqs_cos, freqs_sin, mask, wq, wk, wv, wo, trace=False):
    x = np.asarray(x, dtype=np.float32)
    mask = np.asarray(mask, dtype=np.float32)
    nc, kinds, uniq = _get_compiled(mask)
    in_maps = _host_prep(np.asarray(x), np.asarray(freqs_cos),
                         np.asarray(freqs_sin), mask, np.asarray(wq),
                         np.asarray(wk), np.asarray(wv), np.asarray(wo), uniq)
    res = run_bass_kernel_spmd(nc, in_maps, core_ids=list(range(NC)),
                               trace=trace)
    out = np.concatenate([res.results[c]["out"] for c in range(NC)], axis=0)
    return out.reshape(1, S, D).astype(np.float32), res


def kernel(x, freqs_cos, freqs_sin, mask, wq, wk, wv, wo):
    out, _ = run(x, freqs_cos, freqs_sin, mask, wq, wk, wv, wo, trace=False)
    return out
